# revision 7
# baseline (speedup 1.0000x reference)
"""Masked multi-head self-attention on 8 Trainium2 NeuronCores — v3.

Math per batch element (faithful to the reference up to fp rounding):
    qh = blockdiag(WqT) @ Q (fp32r) ; kh likewise ; vT per-l-block conv (bf16)
    split qh = q1 + q2 (bf16 high + bf16 residual), kh = k1 + k2
    logitsT[h][j, i] = k1.T q1  +  [k1; k2].T [q2; q1]     (2 matmuls, 3 terms)
    P~T[h][j, i]     = exp(logitsT + logm[j])              (mask folded into bias)
    val[h][c, i]     = sum_j vT[h][j, c] P~T[h][j, i]      (+ ones col -> n[i])
    valsc            = val * (1/n broadcast over c)
    out[l, d]        = (valsc.T Wp.T + bp) * mask[l]

Schedule: data-parallel over batch (BS == 8 == n_cores). The attention is
ACT(exp)-paced at ~2.1us per (g, ih, j) superstep; supersteps keep j inner
so each (g, ih) query-half drains while the next half runs; drain PE work is
deferred into the next half's superstep stream so the PE FIFO never stalls.
Startup DMAs are spread across the SP / ACT / GPSIMD DGE queues.
"""

import numpy as np

import concourse.bass as bass
import concourse.mybir as mybir
import concourse.tile as tile
from concourse.bass_utils import run_bass_kernel_spmd
from concourse.vector_clock import ScopedClock

# Problem shapes (hardcoded per contract).
BS, D, L, H = 8, 256, 1024, 8
DK = D // H            # 32
G = 2                  # channel groups of 128 (4 heads each)
JB = L // 128          # 8 key-position blocks
LB = L // 128          # 8 query-position blocks
NEG_BIG = -30000.0     # exp(x + NEG_BIG) == 0 for any realistic logit x
SHIFT = 20.0           # global exp shift: P~ and n scale by e^-SHIFT, P unchanged
F32 = mybir.dt.float32
F32R = mybir.dt.float32r
BF16 = mybir.dt.bfloat16
VP = 34                # vones pitch: [v(32) | ones | pad], 4B-aligned in bf16

_CACHED = {}


def _patch_tile_drain():
    """walrus in this container rejects >1 sync wait on a TPB_CTRL Drain.
    Split the TileContext exit drain's waits across multiple drains."""
    if getattr(tile.TileContext, "_drain_patched", False):
        return

    def _drain_and_barrier(self, tick_clock, wait_clock):
        drain_inst = self.nc.sync.drain(fusable=False)
        wait_clock.add_sem_waits(
            drain_inst.ins, ScopedClock({None: tick_clock.global_clock})
        )
        si = drain_inst.ins.sync_info
        waits = list(si.on_wait or []) if si else []
        if len(waits) > 1:
            si.on_wait = waits[:1]
            drain_inst.ins.sync_info = si
            for w in waits[1:]:
                d2 = self.nc.sync.drain(fusable=False)
                d2.ins.sync_info = mybir.SyncInfo(on_wait=[w], on_update=[])
        self.nc.all_engine_barrier()
        assert self.sems is not None
        popped = self.nc._tile_sem_poison_stack.pop()
        assert popped is self._sem_poison
        self.nc.clear_and_free_semaphores(list(self.sems.allocated().values()))
        self.nc.all_engine_barrier()

    tile.TileContext._drain_and_barrier = _drain_and_barrier
    tile.TileContext._drain_patched = True


def _split_multi_waits(nc, cap=1):
    """This container's walrus accepts at most `cap` sync-wait commands per
    instruction. Hoist extra waits onto same-engine NoOps inserted directly
    before the instruction (engine queues are FIFO, so semantics are
    unchanged)."""
    k = 0
    for fn in nc.m.functions:
        for bb in fn.blocks:
            out = []
            for inst in bb.instructions:
                si = inst.sync_info
                waits = list(si.on_wait) if (si and si.on_wait) else []
                if len(waits) > cap:
                    for i in range(cap, len(waits), cap):
                        nop = mybir.InstNoOp(
                            name=f"waitnop-{k}", engine=inst.engine, ins=[],
                            outs=[],
                            sync_info=mybir.SyncInfo(
                                on_wait=waits[i:i + cap], on_update=[]),
                        )
                        k += 1
                        out.append(nop)
                    si.on_wait = waits[:cap]
                    inst.sync_info = si
                out.append(inst)
            bb.instructions = out


def _build_nc():
    _patch_tile_drain()
    nc = bass.Bass()

    q_d = nc.declare_dram_parameter("q", [D, L], F32R, isOutput=False)
    k_d = nc.declare_dram_parameter("k", [D, L], F32R, isOutput=False)
    v_d = nc.declare_dram_parameter("v", [D, L], BF16, isOutput=False)
    mcols_d = nc.declare_dram_parameter("mcols", [128, JB], F32, isOutput=False)
    sel_d = nc.declare_dram_parameter("sel", [2, 256], F32R, isOutput=False)
    wq_d = nc.declare_dram_parameter("wq", [G, 128, 128], F32R, isOutput=False)
    wk_d = nc.declare_dram_parameter("wk", [G, 128, 128], F32R, isOutput=False)
    wv_d = nc.declare_dram_parameter("wv", [G, 128, 128], BF16, isOutput=False)
    wpt_d = nc.declare_dram_parameter("wpt", [G, 128, D], BF16, isOutput=False)
    bp_d = nc.declare_dram_parameter("bp", [1, D], BF16, isOutput=False)
    out_d = nc.declare_dram_parameter("out", [L, D], F32, isOutput=True)

    EXP = mybir.ActivationFunctionType.Exp
    COPY = mybir.ActivationFunctionType.Copy

    with tile.TileContext(nc) as tc:
        with tc.tile_pool(name="persist", bufs=1) as pp:
            # ---- persistent SBUF tiles -------------------------------------
            qin = [pp.tile([128, L], F32R, tag=f"qin{g}", name=f"qin{g}")
                   for g in range(G)]
            kin = [pp.tile([128, L], F32R, tag=f"kin{g}", name=f"kin{g}")
                   for g in range(G)]
            vinb = [pp.tile([128, L], BF16, tag=f"vinb{g}", name=f"vinb{g}")
                    for g in range(G)]
            wq_t = [pp.tile([128, 128], F32R, tag=f"wq{g}", name=f"wq{g}")
                    for g in range(G)]
            wk_t = [pp.tile([128, 128], F32R, tag=f"wk{g}", name=f"wk{g}")
                    for g in range(G)]
            wv_t = [pp.tile([128, 128], BF16, tag=f"wv{g}", name=f"wv{g}")
                    for g in range(G)]
            wpt_t = [pp.tile([128, D], BF16, tag=f"wpt{g}", name=f"wpt{g}")
                     for g in range(G)]
            bp_t = pp.tile([1, D], BF16, tag="bp", name="bp")
            ones_row = pp.tile([1, 128], BF16, tag="ones_row", name="ones_row")
            sel_t = pp.tile([2, 256], F32R, tag="sel", name="sel")
            mcols_t = pp.tile([128, JB], F32, tag="mcols", name="mcols")
            logm_t = pp.tile([128, JB], F32, tag="logm", name="logm")
            # bf16 splits: x = x1 + x2; term A uses x1 strips, the cross
            # tiles xk/xq pack [k1_h; k2_h] / [q2_h; q1_h] per head pair
            q1a = [pp.tile([128, L], BF16, tag=f"q1a{g}", name=f"q1a{g}")
                   for g in range(G)]
            k1a = [pp.tile([128, L], BF16, tag=f"k1a{g}", name=f"k1a{g}")
                   for g in range(G)]
            q2a = [pp.tile([128, L], BF16, tag=f"q2a{g}", name=f"q2a{g}")
                   for g in range(G)]
            k2a = [pp.tile([128, L], BF16, tag=f"k2a{g}", name=f"k2a{g}")
                   for g in range(G)]
            # [v_head | 1 | pad] stacks: per (group, jblk), bf16 for PV
            vones = [[pp.tile([128, 4 * VP], BF16, tag=f"vo{g}_{j}",
                              name=f"vo{g}_{j}") for j in range(JB)]
                     for g in range(G)]
            valsc = [pp.tile([128, L], BF16, tag=f"valsc{g}", name=f"valsc{g}")
                     for g in range(G)]          # normalized val, proj input
            nmt = [[pp.tile([2, L], F32, tag=f"nm{g}_{p}", name=f"nm{g}_{p}")
                    for p in range(2)] for g in range(G)]  # softmax sums
            nrow = [pp.tile([1, 512], F32, tag=f"nrow{h}", name=f"nrow{h}")
                    for h in range(2)]           # aligned n bounce (per drain)
            rmt = [[pp.tile([2, L], F32R, tag=f"rm{g}_{p}", name=f"rm{g}_{p}")
                    for p in range(2)] for g in range(G)]  # 1/n rows
            rps = pp.tile([64, 512], F32R, tag="rps", name="rps")

            # ---- loads, spread across the SP / ACT DGE queues --------------
            # (gpsimd software DGE is slow -- keep it off the critical path)
            nc.sync.dma_start(mcols_t[:], mcols_d[:])
            nc.sync.dma_start(sel_t[:], sel_d[:])
            nc.sync.dma_start(wk_t[0][:], wk_d[0])
            nc.sync.dma_start(wv_t[0][:], wv_d[0])
            nc.scalar.dma_start(wq_t[0][:], wq_d[0])
            nc.sync.dma_start(kin[0][:], k_d[0:128, :])
            nc.scalar.dma_start(qin[0][:], q_d[0:128, :])
            nc.sync.dma_start(vinb[0][:], v_d[0:128, :])
            nc.sync.dma_start(wk_t[1][:], wk_d[1])
            nc.scalar.dma_start(wq_t[1][:], wq_d[1])
            nc.sync.dma_start(kin[1][:], k_d[128:256, :])
            nc.scalar.dma_start(qin[1][:], q_d[128:256, :])
            nc.sync.dma_start(wv_t[1][:], wv_d[1])
            nc.sync.dma_start(vinb[1][:], v_d[128:256, :])
            nc.scalar.dma_start(wpt_t[0][:], wpt_d[0])
            nc.scalar.dma_start(wpt_t[1][:], wpt_d[1])
            nc.scalar.dma_start(bp_t[:], bp_d[:])
            nc.vector.memset(ones_row[:], 1.0)
            # logmask columns: (m - 1) * |NEG_BIG| - SHIFT  ->  -SHIFT or ~NEG_BIG
            nc.scalar.activation(logm_t[:], mcols_t[:], COPY,
                                 bias=NEG_BIG - SHIFT, scale=-NEG_BIG)

            # ---- phase A: fp32r convs, bf16 splits, cross-tile assembly ----
            with tc.tile_pool(name="cpsum", bufs=2, space="PSUM") as cps, \
                 tc.tile_pool(name="vtpsum", bufs=2, space="PSUM") as vps:
                for g in range(G):
                    for nm_in, w_t, hi_t, lo_t in (
                            (kin[g], wk_t[g], k1a[g], k2a[g]),
                            (qin[g], wq_t[g], q1a[g], q2a[g])):
                        cp = cps.tile([128, L], F32, tag="convp", name="convp")
                        for ihalf in range(2):
                            nc.tensor.matmul(
                                cp[:, 512 * ihalf:512 * (ihalf + 1)], w_t[:],
                                nm_in[:, 512 * ihalf:512 * (ihalf + 1)],
                                start=True, stop=True)
                        nc.vector.tensor_copy(hi_t[:], cp[:])
                        nc.vector.tensor_sub(lo_t[:], cp[:], hi_t[:])
                    # vT: per (g, lblk): V_g[:, lblk].T @ blockdiag(WvT)
                    for j in range(JB):
                        vp = vps.tile([128, 128], F32, tag="vtp", name="vtp")
                        nc.tensor.matmul(vp[:], vinb[g][:, 128 * j:128 * (j + 1)],
                                         wv_t[g][:], start=True, stop=True)
                        vo = vones[g][j]
                        vo3 = vo.rearrange("p (h c) -> p h c", c=VP)
                        nc.vector.memset(vo3[:, :, DK:DK + 1], 1.0)
                        vp3 = vp.rearrange("p (h c) -> p h c", c=DK)
                        nc.vector.tensor_copy(vo3[:, :, 0:DK], vp3[:])

            # ---- phase B: attention, (g, iq) quarters with j inner ---------
            # One [128, 1024] logits tile per superstep: 4 heads x 256
            # queries, so one exp per superstep and qkt bufs=2 gives two
            # supersteps of pipeline slack (PE runs ahead of ACT).
            IQ = 4
            IW = L // IQ           # 256 queries per superstep
            with tc.tile_pool(name="qkt", bufs=2, space="PSUM") as qkt_pool, \
                 tc.tile_pool(name="valp", bufs=3, space="PSUM") as val_pool, \
                 tc.tile_pool(name="scr", bufs=1, space="PSUM") as scr_pool, \
                 tc.tile_pool(name="pt", bufs=6) as pt_pool, \
                 tc.tile_pool(name="outp", bufs=4) as out_pool:

                pending_pv = None
                deferred_pe = []     # drain PE closures, run 1/superstep

                def emit_pv(val_, pt_, g_, j_):
                    # head hh -> val rows 64*(hh%2), cols 256*(hh//2)
                    for hh in range(4):
                        ro = 64 * (hh % 2)
                        co = IW * (hh // 2)
                        nc.tensor.matmul(
                            val_[ro:ro + DK + 1, co:co + IW],
                            vones[g_][j_][:, VP * hh:VP * hh + DK + 1],
                            pt_[:, IW * hh:IW * (hh + 1)],
                            start=(j_ == 0), stop=(j_ == JB - 1),
                            skip_group_check=True,
                        )

                def drain_quarter(g_, iq_, val_, last):
                    cs = slice(IW * iq_, IW * (iq_ + 1))
                    # n rows out of PSUM (partition 32/96 of the val tile)
                    for hh in range(4):
                        ro = 64 * (hh % 2) + DK
                        co = IW * (hh // 2)
                        src = val_[ro:ro + 1, co:co + IW]
                        if last and hh >= 2:
                            nc.scalar.activation(nrow[hh][:], src, COPY)
                        else:
                            nc.vector.tensor_copy(nrow[hh][:], src)
                        nc.sync.dma_start(nm[g_][hh:hh + 1, cs], nrow[hh][:])
                    with nc.allow_low_precision(reason="softmax 1/n in f32r is within the error gate"):
                        nc.vector.reciprocal(rm[g_][:, cs], nm[g_][:, cs])

                    def pe_rp():
                        scr = scr_pool.tile([128, 512], F32, tag="scr",
                                            name="scr")
                        rp = scr[:, 0:IW]
                        nc.tensor.matmul(rp, sel_t[:], rm[g_][:, cs],
                                         start=True, stop=True)
                        nc.vector.tensor_copy(rps[:], rp)
                        for hh in range(4):
                            ro = 64 * (hh % 2)
                            co = IW * (hh // 2)
                            nc.vector.tensor_mul(
                                valsc[g_][32 * hh:32 * hh + 32, cs],
                                val_[ro:ro + 32, co:co + IW],
                                rps[32 * hh:32 * hh + 32, :])

                    def make_proj(lb):
                        def pe_proj():
                            scr = scr_pool.tile([128, 512], F32, tag="scr",
                                                name="scr")
                            pj = scr[:, 0:D]
                            ls = slice(128 * lb, 128 * (lb + 1))
                            nc.tensor.matmul(pj, valsc[0][:, ls], wpt_t[0][:],
                                             start=True, stop=False)
                            nc.tensor.matmul(pj, valsc[1][:, ls], wpt_t[1][:],
                                             start=False, stop=False)
                            nc.tensor.matmul(pj, ones_row[:], bp_t[:],
                                             start=False, stop=True)
                            ot = out_pool.tile([128, D], F32, tag="ot",
                                               name="ot")
                            if last:
                                nc.scalar.activation(
                                    ot[:], pj, COPY,
                                    scale=mcols_t[:, lb:lb + 1])
                            else:
                                nc.vector.tensor_scalar_mul(
                                    ot[:], pj, mcols_t[:, lb:lb + 1])
                            nc.scalar.dma_start(out_d[ls, :], ot[:])
                        return pe_proj

                    steps = [pe_rp]
                    if g_ == G - 1:
                        steps.append(make_proj(2 * iq_))
                        steps.append(make_proj(2 * iq_ + 1))
                    if last:
                        for s in steps:
                            s()
                    else:
                        deferred_pe.extend(steps)

                for g in range(G):
                    for iq in range(IQ):
                        val = val_pool.tile([128, 512], F32, tag="val",
                                            name="val")
                        for j in range(JB):
                            lo = qkt_pool.tile([128, L], F32, tag="lo",
                                               name="lo")
                            js = slice(128 * j, 128 * (j + 1))
                            is_ = slice(IW * iq, IW * (iq + 1))
                            # term A: 4 k1.T q1 strips (distinct row groups)
                            for hh in range(4):
                                ps = slice(32 * hh, 32 * (hh + 1))
                                nc.tensor.matmul(
                                    lo[:, IW * hh:IW * (hh + 1)],
                                    k1a[g][ps, js], q1a[g][ps, is_],
                                    start=True, stop=False,
                                    tile_position=(32 * hh, 0),
                                    skip_group_check=True,
                                )
                            # cross: [k1;k2].T [q2;q1] per head, K=64 strips
                            for hh in range(4):
                                ps = slice(64 * (hh % 2), 64 * (hh % 2) + 64)
                                nc.tensor.matmul(
                                    lo[:, IW * hh:IW * (hh + 1)],
                                    kx[g][hh // 2][ps, js],
                                    qx[g][hh // 2][ps, is_],
                                    start=False, stop=True,
                                    tile_position=(64 * (hh % 2), 0),
                                    skip_group_check=True,
                                )
                            # deferred drain PE work MUST precede the pending
                            # PV in the PE FIFO: the PV may wait on a val
                            # buffer that only frees once the drain muls (fed
                            # by the deferred rp matmul) complete
                            if deferred_pe and j >= 1:
                                deferred_pe.pop(0)()
                            if pending_pv is not None:
                                emit_pv(*pending_pv)
                                pending_pv = None
                            pt = pt_pool.tile([128, L], BF16, tag="pt",
                                              name="pt")
                            nc.scalar.activation(pt[:], lo[:], EXP,
                                                 bias=logm_t[:, j:j + 1])
                            pending_pv = (val, pt, g, j)
                        emit_pv(*pending_pv)
                        pending_pv = None
                        drain_quarter(g, iq, val,
                                      last=(g == G - 1 and iq == IQ - 1))
                while deferred_pe:
                    deferred_pe.pop(0)()

    _split_multi_waits(nc)
    return nc


def _host_prep(queries, keys, values, mask, Wq, Wk, Wv, Wp, bp):
    """Shared (per-core-invariant) weight tensors + per-core input maps."""
    f32 = np.float32

    import ml_dtypes

    def bdT(W, g):
        out = np.zeros((128, 128), f32)
        for j in range(4):
            out[32 * j:32 * (j + 1), 32 * j:32 * (j + 1)] = W[4 * g + j].T
        return out

    wq = np.stack([bdT(Wq, g) for g in range(G)]).astype(f32)
    wk = np.stack([bdT(Wk, g) for g in range(G)]).astype(f32)
    wv = np.stack([bdT(Wv, g) for g in range(G)]).astype(ml_dtypes.bfloat16)
    wpt = np.ascontiguousarray(Wp.T.reshape(G, 128, D)).astype(ml_dtypes.bfloat16)
    bpr = np.asarray(bp).reshape(1, D).astype(ml_dtypes.bfloat16)
    sel = np.zeros((2, 256), f32)
    for hp in range(2):
        for w in range(2):
            m0 = 32 * (2 * hp + w)
            sel[w, 128 * hp + m0:128 * hp + m0 + 32] = 1.0

    in_maps = []
    for b in range(BS):
        m = np.ascontiguousarray(mask[b, :, 0]).astype(f32)
        in_maps.append({
            "q": np.ascontiguousarray(queries[b]).astype(f32),
            "k": np.ascontiguousarray(keys[b]).astype(f32),
            "v": np.ascontiguousarray(values[b]).astype(ml_dtypes.bfloat16),
            "mcols": np.ascontiguousarray(m.reshape(JB, 128).T).astype(f32),
            "sel": sel,
            "wq": wq, "wk": wk, "wv": wv, "wpt": wpt, "bp": bpr,
        })
    return in_maps


def _run(in_maps, **kwargs):
    if "nc" not in _CACHED:
        _CACHED["nc"] = _build_nc()
    return run_bass_kernel_spmd(_CACHED["nc"], in_maps, list(range(BS)), **kwargs)


def kernel(queries, keys, values, mask, Wq, Wk, Wv, Wp, bp):
    in_maps = _host_prep(queries, keys, values, mask, Wq, Wk, Wv, Wp, bp)
    res = _run(in_maps)
    return np.stack([res.results[b]["out"] for b in range(BS)]).astype(np.float32)


# revision 9
# speedup vs baseline: 1.4406x; 1.4406x over previous
"""Masked multi-head self-attention on 8 Trainium2 NeuronCores.

Math (per batch element b, faithful to the reference up to fp rounding):
    q = blockdiag(Wq) @ Q ; k = blockdiag(Wk) @ K ; vT = Q-style grouped conv,
    logitsT[h][j, i] = sum_c k[h][c, j] * q[h][c, i]        (j = key pos, i = query pos)
    P~T[h][j, i]    = exp(logitsT + logmask[j])             (mask folded into exp bias;
                                                             softmax max-shift dropped --
                                                             logits are O(40), exp is safe)
    val[h][c, i]    = sum_j vT[h][j, c] * P~T[h][j, i]      (plus a ones column giving
                                                             n[i] = sum_j P~T[j, i])
    val_scaled      = val * (mask[i] / n[i])                 (per-head normalizer)
    outT[l, d]      = sum_j val_scaled[j, l] * WpT[j, d] + mask[l] * bp[d]

Sharding: pure data-parallel over batch (BS == 8 == n_cores), no collectives.
"""

import numpy as np

import concourse.bass as bass
import concourse.mybir as mybir
import concourse.tile as tile
from concourse.bass_utils import run_bass_kernel_spmd
from concourse.vector_clock import ScopedClock

# Problem shapes (hardcoded per contract).
BS, D, L, H = 8, 256, 1024, 8
DK = D // H            # 32
G = 2                  # channel groups of 128 (4 heads each)
JB = L // 128          # 8 key-position blocks
LB = L // 128          # 8 query-position blocks
NEG_BIG = -30000.0     # exp(x + NEG_BIG) == 0 for any realistic logit x
SHIFT = 20.0           # global exp shift: P~ and n scale by e^-SHIFT, P unchanged;
                       # guards fp32 overflow for logits up to ~108
F32 = mybir.dt.float32
BF16 = mybir.dt.bfloat16
VP = 34                # vones pitch: [v(32) | ones | pad], 4B-aligned in bf16

_CACHED = {}


def _patch_tile_drain():
    """walrus in this container rejects >1 sync wait on a TPB_CTRL Drain.
    Split the TileContext exit drain's waits across multiple drains."""
    if getattr(tile.TileContext, "_drain_patched", False):
        return

    def _drain_and_barrier(self, tick_clock, wait_clock):
        drain_inst = self.nc.sync.drain(fusable=False)
        wait_clock.add_sem_waits(
            drain_inst.ins, ScopedClock({None: tick_clock.global_clock})
        )
        si = drain_inst.ins.sync_info
        waits = list(si.on_wait or []) if si else []
        if len(waits) > 1:
            si.on_wait = waits[:1]
            drain_inst.ins.sync_info = si
            for w in waits[1:]:
                d2 = self.nc.sync.drain(fusable=False)
                d2.ins.sync_info = mybir.SyncInfo(on_wait=[w], on_update=[])
        self.nc.all_engine_barrier()
        assert self.sems is not None
        popped = self.nc._tile_sem_poison_stack.pop()
        assert popped is self._sem_poison
        self.nc.clear_and_free_semaphores(list(self.sems.allocated().values()))
        self.nc.all_engine_barrier()

    tile.TileContext._drain_and_barrier = _drain_and_barrier
    tile.TileContext._drain_patched = True


def _split_multi_waits(nc, cap=1):
    """This container's walrus accepts at most `cap` sync-wait commands per
    instruction. Hoist extra waits onto same-engine NoOps inserted directly
    before the instruction (engine queues are FIFO, so semantics are
    unchanged)."""
    k = 0
    for fn in nc.m.functions:
        for bb in fn.blocks:
            out = []
            for inst in bb.instructions:
                si = inst.sync_info
                waits = list(si.on_wait) if (si and si.on_wait) else []
                if len(waits) > cap:
                    for i in range(cap, len(waits), cap):
                        nop = mybir.InstNoOp(
                            name=f"waitnop-{k}", engine=inst.engine, ins=[],
                            outs=[],
                            sync_info=mybir.SyncInfo(
                                on_wait=waits[i:i + cap], on_update=[]),
                        )
                        k += 1
                        out.append(nop)
                    si.on_wait = waits[:cap]
                    inst.sync_info = si
                out.append(inst)
            bb.instructions = out


def _build_nc(repeat=1, skip=()):
    _patch_tile_drain()
    nc = bass.Bass()

    q_d = nc.declare_dram_parameter("q", [D, L], F32, isOutput=False)
    k_d = nc.declare_dram_parameter("k", [D, L], F32, isOutput=False)
    v_d = nc.declare_dram_parameter("v", [D, L], BF16, isOutput=False)
    mcols_d = nc.declare_dram_parameter("mcols", [128, JB], F32, isOutput=False)
    sel_d = nc.declare_dram_parameter("sel", [4, 128], BF16, isOutput=False)
    wq_d = nc.declare_dram_parameter("wq", [G, 128, 128], F32, isOutput=False)
    wk_d = nc.declare_dram_parameter("wk", [G, 128, 128], F32, isOutput=False)
    wv_d = nc.declare_dram_parameter("wv", [G, 128, 128], BF16, isOutput=False)
    wpt_d = nc.declare_dram_parameter("wpt", [G, 128, D], BF16, isOutput=False)
    bp_d = nc.declare_dram_parameter("bp", [1, D], BF16, isOutput=False)
    out_d = nc.declare_dram_parameter("out", [L, D], F32, isOutput=True)

    EXP = mybir.ActivationFunctionType.Exp
    COPY = mybir.ActivationFunctionType.Copy

    with tile.TileContext(nc) as tc:
        with tc.tile_pool(name="persist", bufs=1) as pp:
            # ---- persistent SBUF tiles -------------------------------------
            def ptile(tag, shape):
                return pp.tile(shape, F32, tag=tag, name=tag)

            qin = [ptile(f"qin{g}", [128, L]) for g in range(G)]
            kin = [ptile(f"kin{g}", [128, L]) for g in range(G)]

            wq_t = [ptile(f"wq{g}", [128, 128]) for g in range(G)]
            wk_t = [ptile(f"wk{g}", [128, 128]) for g in range(G)]
            wv_t = [pp.tile([128, 128], BF16, tag=f"wv{g}", name=f"wv{g}") for g in range(G)]
            wpt_t = [pp.tile([128, D], BF16, tag=f"wpt{g}", name=f"wpt{g}") for g in range(G)]
            bp_t = pp.tile([1, D], BF16, tag="bp", name="bp")
            ones_row = pp.tile([1, 128], BF16, tag="ones_row", name="ones_row")
            sel_t = pp.tile([4, 128], BF16, tag="sel", name="sel")
            mcols_t = ptile("mcols", [128, JB])
            logm_t = ptile("logm", [128, JB])
            qh = [ptile(f"qh{g}", [128, L]) for g in range(G)]       # conv'd q
            kh = [ptile(f"kh{g}", [128, L]) for g in range(G)]       # conv'd k
            # split-bf16 halves of qh/kh: x = x1 + x2 with x1 = bf16(x);
            # logits = k1*q1 + k1*q2 + k2*q1 (+k2*q2 dropped, ~2^-16 rel)
            q1a = [pp.tile([128, L], BF16, tag=f"q1a{g}", name=f"q1a{g}")
                   for g in range(G)]
            q2a = [pp.tile([128, L], BF16, tag=f"q2a{g}", name=f"q2a{g}")
                   for g in range(G)]
            k1a = [pp.tile([128, L], BF16, tag=f"k1a{g}", name=f"k1a{g}")
                   for g in range(G)]
            k2a = [pp.tile([128, L], BF16, tag=f"k2a{g}", name=f"k2a{g}")
                   for g in range(G)]
            scr = ptile("scr", [128, L])                             # residual scratch
            vinb = [pp.tile([128, L], BF16, tag=f"vinb{g}", name=f"vinb{g}")
                    for g in range(G)]
            # [v_head | 1 | pad] stacks: per (group, jblk), bf16 for the PV matmul
            vones = [[pp.tile([128, 4 * VP], BF16, tag=f"vo{g}_{j}",
                              name=f"vo{g}_{j}") for j in range(JB)]
                     for g in range(G)]
            valk = [pp.tile([128, L], BF16, tag=f"valk{g}", name=f"valk{g}") for g in range(G)]   # raw val (bf16), K-tile layout
            valsc = [pp.tile([128, L], BF16, tag=f"valsc{g}", name=f"valsc{g}") for g in range(G)]  # normalized val, bf16 for proj
            nm = [ptile(f"nm{g}", [4, L]) for g in range(G)]         # per-head softmax sums
            nrow = [ptile(f"nrow{h}", [1, L]) for h in range(H)]     # aligned n bounce
            rm = [pp.tile([4, L], BF16, tag=f"rm{g}", name=f"rm{g}") for g in range(G)]  # 1/n rows (bf16)
            rsc = [ptile(f"rsc{g}", [4, L]) for g in range(G)]       # recip scratch

            for _rep in range(repeat):
                # ---- load everything -------------------------------------------
                for g in range(G):
                    nc.scalar.dma_start(qin[g][:], q_d[128 * g:128 * (g + 1), :])
                    nc.scalar.dma_start(wq_t[g][:], wq_d[g])
                    nc.sync.dma_start(kin[g][:], k_d[128 * g:128 * (g + 1), :])
                    nc.sync.dma_start(wk_t[g][:], wk_d[g])
                nc.sync.dma_start(mcols_t[:], mcols_d[:])
                for g in range(G):
                    nc.sync.dma_start(vinb[g][:], v_d[128 * g:128 * (g + 1), :])
                    nc.sync.dma_start(wv_t[g][:], wv_d[g])
                for g in range(G):
                    nc.scalar.dma_start(wpt_t[g][:], wpt_d[g])
                nc.scalar.dma_start(bp_t[:], bp_d[:])
                nc.vector.memset(ones_row[:], 1.0)
                nc.sync.dma_start(sel_t[:], sel_d[:])
                # logmask columns: (m - 1) * |NEG_BIG|  ->  0 or NEG_BIG
                nc.scalar.activation(logm_t[:], mcols_t[:], COPY,
                                     bias=NEG_BIG - SHIFT, scale=-NEG_BIG)

                # ---- phase A: grouped 1x1 convs ---------------------------------
                with tc.tile_pool(name=f"qkt{_rep}", bufs=2, space="PSUM") as qkt_pool, \
                     tc.tile_pool(name=f"valp{_rep}", bufs=2, space="PSUM") as val_pool, \
                     tc.tile_pool(name=f"pt{_rep}", bufs=6) as pt_pool:
                    for g in range(G):
                        qp = qkt_pool.tile([128, L], F32, tag="lo", name="lo")
                        for ih in range(2):
                            nc.tensor.matmul(qp[:, 512 * ih:512 * (ih + 1)], wq_t[g][:],
                                             qin[g][:, 512 * ih:512 * (ih + 1)])
                        nc.vector.tensor_copy(qh[g][:], qp[:])
                        kp = qkt_pool.tile([128, L], F32, tag="lo", name="lo")
                        for ih in range(2):
                            nc.tensor.matmul(kp[:, 512 * ih:512 * (ih + 1)], wk_t[g][:],
                                             kin[g][:, 512 * ih:512 * (ih + 1)])
                        nc.vector.tensor_copy(kh[g][:], kp[:])
                    # split qh/kh into bf16 high + bf16 residual
                    for g in range(G):
                        for full, hi_t, lo_t in ((qh[g], q1a[g], q2a[g]),
                                                 (kh[g], k1a[g], k2a[g])):
                            nc.vector.tensor_copy(hi_t[:], full[:])
                            nc.vector.tensor_sub(scr[:], full[:], hi_t[:])
                            nc.vector.tensor_copy(lo_t[:], scr[:])
                    # vT: per (g, lblk): (128 l x 128 heads*dk) = V_g[:, lblk].T @ blockdiag(WvT)
                    for g in range(G):
                        for j in range(JB):
                            vpt = val_pool.tile([128, L], F32, tag="val", name="val")
                            nc.tensor.matmul(vpt[:, 0:128],
                                             vinb[g][:, 128 * j:128 * (j + 1)],
                                             wv_t[g][:])
                            vo = vones[g][j]
                            vo3 = vo.rearrange("p (h c) -> p h c", c=VP)
                            nc.vector.memset(vo3[:, :, DK:DK + 1], 1.0)
                            vp3 = vpt[:, 0:128].rearrange("p (h c) -> p h c", c=DK)
                            nc.vector.tensor_copy(vo3[:, :, 0:DK], vp3)

                # ---- phase B: attention, one 4-head group at a time -------------
                # Superstep (g, j, ihalf): two PSUM tiles each holding two heads'
                # logitsT slices -> 4 QKT matmuls on distinct 32-row PE strips
                # (concurrent on HW) -> one exp per tile (FD=1024, bf16 out) ->
                # 4 bf16 PV matmuls (col-paired, M=33 incl. the n ones-column).
                # (pools shared with phase A)
                    group_vals = []
                    pending_pv = None

                    def _drain_group(g_, vals_):
                        tail = g_ == G - 1
                        for pr in range(2):
                            for hi in range(2):
                                h = 4 * g_ + 2 * pr + hi
                                qoff = 64 * hi
                                co = 32 * (h % 4)
                                nc.vector.tensor_copy(valk[g_][co:co + 32, :],
                                                      vals_[pr][qoff:qoff + 32, :])
                                if tail:
                                    # ACT is idle once attention ends; keep the
                                    # critical tail chain off the busy DVE
                                    nc.scalar.activation(
                                        nrow[h][:],
                                        vals_[pr][qoff + 32:qoff + 33, :], COPY)
                                else:
                                    nc.vector.tensor_copy(
                                        nrow[h][:],
                                        vals_[pr][qoff + 32:qoff + 33, :])
                                nc.sync.dma_start(nm[g_][h % 4:h % 4 + 1, :],
                                                  nrow[h][:])
                        with nc.allow_low_precision(reason="softmax 1/n in bf16 is within the error gate"):
                            nc.vector.reciprocal(rm[g_][:], nm[g_][:])

                    def emit_pv(vals_, pts_, g_, j_, ih_):
                        for pr in range(2):
                            for hi in range(2):
                                hh = 2 * pr + hi
                                nc.tensor.matmul(
                                    vals_[pr][64 * hi:64 * hi + DK + 1,
                                              512 * ih_:512 * (ih_ + 1)],
                                    vones[g_][j_][:, VP * hh:VP * hh + DK + 1],
                                    pts_[pr][:, 512 * hi:512 * (hi + 1)],
                                    start=(j_ == 0), stop=(j_ == JB - 1),
                                    skip_group_check=True,
                                )

                    for g in range(G):
                        vals = [val_pool.tile([128, L], F32, tag="val", name="val")
                                for _ in range(2)]
                        group_vals.append(vals)
                        for j in range(JB):
                            for ih in range(2):
                                pts = []
                                los = []
                                for pr in range(2):          # head pairs (0,1),(2,3)
                                    lo = qkt_pool.tile([128, L], F32, tag="lo",
                                                       name="lo")
                                    los.append(lo)
                                    for hi in range(2):
                                        hh = 2 * pr + hi
                                        ps = slice(32 * hh, 32 * (hh + 1))
                                        js = slice(128 * j, 128 * (j + 1))
                                        is_ = slice(512 * ih, 512 * (ih + 1))
                                        terms = ((k1a[g], q1a[g]),
                                                 (k1a[g], q2a[g]),
                                                 (k2a[g], q1a[g]))
                                        for ti, (kt_, qt_) in enumerate(terms):
                                            nc.tensor.matmul(
                                                lo[:, 512 * hi:512 * (hi + 1)],
                                                kt_[ps, js], qt_[ps, is_],
                                                start=(ti == 0), stop=(ti == 2),
                                                tile_position=(32 * hh, 0),
                                                skip_group_check=True,
                                            )
                                # previous superstep's PV lands on the PE queue
                                # here, between this superstep's QKT and the
                                # next one's, so PE never stalls waiting on exp
                                if pending_pv is not None:
                                    emit_pv(*pending_pv)
                                for pr in range(2):
                                    pt = pt_pool.tile([128, L], BF16, tag="pt",
                                                      name="pt")
                                    nc.scalar.activation(pt[:], los[pr][:], EXP,
                                                         bias=logm_t[:, j:j + 1])
                                    pts.append(pt)
                                pending_pv = (vals, pts, g, j, ih)
                        if g + 1 < G:
                            # flush group g's last PV now so its drain can
                            # overlap group g+1's supersteps
                            emit_pv(*pending_pv)
                            pending_pv = None
                            _drain_group(g, vals)
                    emit_pv(*pending_pv)
                    pending_pv = None
                    _drain_group(G - 1, group_vals[G - 1])

                # ---- phase C: normalizers + scaling -----------------------------
                with tc.tile_pool(name=f"rpsum{_rep}", bufs=1, space="PSUM") as rps:
                    for g in range(G):
                        rp = rps.tile([128, L], F32, tag="rp", name="rp")
                        for ih in range(2):
                            nc.tensor.matmul(rp[:, 512 * ih:512 * (ih + 1)], sel_t[:],
                                             rm[g][:, 512 * ih:512 * (ih + 1)])
                        nc.vector.tensor_mul(valsc[g][:], valk[g][:], rp[:])

                # ---- phase D: projection + bias + mask + store ------------------
                with tc.tile_pool(name=f"projp{_rep}", bufs=4, space="PSUM") as pjp, \
                     tc.tile_pool(name=f"outp{_rep}", bufs=4) as outp:
                    for lb in range(LB):
                        ls = slice(128 * lb, 128 * (lb + 1))
                        pj = pjp.tile([128, D], F32, tag="pj", name="pj")
                        nc.tensor.matmul(pj[:], valsc[0][:, ls], wpt_t[0][:],
                                         start=True, stop=False)
                        nc.tensor.matmul(pj[:], valsc[1][:, ls], wpt_t[1][:],
                                         start=False, stop=False)
                        nc.tensor.matmul(pj[:], ones_row[:], bp_t[:],
                                         start=False, stop=True)
                        ot = outp.tile([128, D], F32, tag="ot", name="ot")
                        nc.scalar.activation(ot[:], pj[:], COPY,
                                             scale=mcols_t[:, lb:lb + 1])
                        nc.sync.dma_start(out_d[ls, :], ot[:])

    _split_multi_waits(nc)
    return nc


def _host_prep(queries, keys, values, mask, Wq, Wk, Wv, Wp, bp):
    """Shared (per-core-invariant) weight tensors + per-core input maps."""
    f32 = np.float32

    import ml_dtypes

    def bdT(W, g):
        out = np.zeros((128, 128), f32)
        for j in range(4):
            out[32 * j:32 * (j + 1), 32 * j:32 * (j + 1)] = W[4 * g + j].T
        return out

    wq = np.stack([bdT(Wq, g) for g in range(G)]).astype(f32)
    wk = np.stack([bdT(Wk, g) for g in range(G)]).astype(f32)
    wv = np.stack([bdT(Wv, g) for g in range(G)]).astype(ml_dtypes.bfloat16)
    wpt = np.ascontiguousarray(Wp.T.reshape(G, 128, D)).astype(ml_dtypes.bfloat16)
    bpr = np.asarray(bp).reshape(1, D).astype(ml_dtypes.bfloat16)
    sel = np.zeros((4, 128), ml_dtypes.bfloat16)
    for a in range(4):
        sel[a, 32 * a:32 * (a + 1)] = 1.0

    in_maps = []
    for b in range(BS):
        m = np.ascontiguousarray(mask[b, :, 0]).astype(f32)
        in_maps.append({
            "q": np.ascontiguousarray(queries[b]).astype(f32),
            "k": np.ascontiguousarray(keys[b]).astype(f32),
            "v": np.ascontiguousarray(values[b]).astype(ml_dtypes.bfloat16),
            "mcols": np.ascontiguousarray(m.reshape(JB, 128).T).astype(f32),
            "sel": sel,
            "wq": wq, "wk": wk, "wv": wv, "wpt": wpt, "bp": bpr,
        })
    return in_maps


def _run(in_maps, **kwargs):
    if "nc" not in _CACHED:
        _CACHED["nc"] = _build_nc()
    return run_bass_kernel_spmd(_CACHED["nc"], in_maps, list(range(BS)), **kwargs)


def kernel(queries, keys, values, mask, Wq, Wk, Wv, Wp, bp):
    in_maps = _host_prep(queries, keys, values, mask, Wq, Wk, Wv, Wp, bp)
    res = _run(in_maps)
    return np.stack([res.results[b]["out"] for b in range(BS)]).astype(np.float32)

